# revision 41
# baseline (speedup 1.0000x reference)
"""Graphormer multi-head attention on 8 TRN2 NeuronCores.

Sharding (2D, data + head parallel): core c -> batch c//4, head-quad c%4
(4 heads per core as 2 pairs).  Per-core DMA: q/k/v only for its batch
(12.6 MB), bias slice 33.5 MB bf16, bf16 partial output 4.2 MB.

v1 design (vs the 292 us baseline):
 - t-blocks of 512 (8 blocks: (p0,th0),(p0,th1),(p1,th0),(p1,th1),
   (p0,th2),(p1,th2),(p0,th3),(p1,th3)).
 - scores per (head, st) land in a [128, 2, 512] fp32 PSUM tile (2 banks,
   ping-ponged pool of 2 => 4 banks); ONE exp per st covers both heads
   (FD=1024, amortizing the ~352-cycle ACT per-instruction overhead:
   256x512-elem exps @720ns -> 128x1024-elem @1147ns, ~45 us saved).
 - PV accumulates into ONE [65, 2, 512] fp32 tile (2 banks, ones row at
   partition 64 for the softmax denominator); PV emission lags scores by
   LAG st-tiles so the previous block's normalization can drain pouts.
 - remaining 2 PSUM banks are the weave pool (QKV projection chunks and
   out-projection tiles interleave into the st-loops).
 - V transposes moved off the PE: SBUF->SBUF xbar dma_start_transpose to
   a [128, 4, 128] scratch (row r -> partition r%128, tile r//128), then
   one strided 2x DVE copy into vnat.
 - DMA split across 3 queues: bias stream -> sync (SP), x/weights (+
   transposes) -> scalar, output stores -> gpsimd SWDGE.  Baseline pushed
   all 52 MB through the single sync queue at ~180 GB/s average.
 - host x layout [128, chunk, EC, 512] so each x DMA is 1 MB with 8 KB
   per-partition contiguous lines.
"""

import os
from contextlib import ExitStack

import ml_dtypes
import numpy as np

import concourse.bass as bass
import concourse.tile as tile
from concourse import bacc
from concourse import mybir
from concourse.bass_utils import run_bass_kernel_spmd

B, T, S, E, H, D = 2, 2048, 2048, 1024, 16, 64
NCORES = 8
HPC = 4                    # heads per core
NPAIR = 2                  # head pairs per core
PSL = HPC * D              # per-core projection slice = 256
EC = E // 128              # contraction chunks = 8
ST = S // 128              # s tiles = 16
TCH = 512                  # t block
NTH = T // TCH             # 4
NCH = T // TCH             # x column chunks = 4
NB = 512                   # fp32 psum bank free size
LAG = 5                    # PV lags scores by this many st tiles
BF16 = mybir.dt.bfloat16
F32 = mybir.dt.float32
NPBF16 = ml_dtypes.bfloat16
AF = mybir.ActivationFunctionType

_MODULES = {}
LAST_RUN = None


def build_module():
    key = "main"
    if key in _MODULES:
        return _MODULES[key]

    nc = bacc.Bacc("TRN2", target_bir_lowering=False, debug=False)

    qT_d = nc.dram_tensor("qT", [128, NCH, EC, TCH], BF16, kind="ExternalInput")
    kT_d = nc.dram_tensor("kT", [128, NCH, EC, TCH], BF16, kind="ExternalInput")
    vT_d = nc.dram_tensor("vT", [128, NCH, EC, TCH], BF16, kind="ExternalInput")
    # host layout: [th, pair, sp2, 128, j(st in pair), h(in pair), t]
    bias_d = nc.dram_tensor("biasT", [NTH, NPAIR, ST // 2, 128, 2, 2, TCH],
                            BF16, kind="ExternalInput")
    wq_d = nc.dram_tensor("wqT", [128, EC, PSL], BF16, kind="ExternalInput")
    wk_d = nc.dram_tensor("wkT", [128, EC, PSL], BF16, kind="ExternalInput")
    wv_d = nc.dram_tensor("wvT", [128, EC, PSL], BF16, kind="ExternalInput")
    wo_d = nc.dram_tensor("woT", [128, NPAIR, E], BF16, kind="ExternalInput")
    bq_d = nc.dram_tensor("bq", [128, NPAIR], F32, kind="ExternalInput")
    bk_d = nc.dram_tensor("bk", [128, NPAIR], F32, kind="ExternalInput")
    bv_d = nc.dram_tensor("bv", [128, NPAIR], F32, kind="ExternalInput")
    y_d = nc.dram_tensor("ypart", [T, E], BF16, kind="ExternalOutput")

    with tile.TileContext(nc) as tc, ExitStack() as ctx:
        consts = ctx.enter_context(tc.tile_pool(name="consts", bufs=1))
        xpool = ctx.enter_context(tc.tile_pool(name="xstage", bufs=1))
        persist = ctx.enter_context(tc.tile_pool(name="persist", bufs=1))
        biasp = ctx.enter_context(tc.tile_pool(name="biasp", bufs=4))
        ptp = ctx.enter_context(tc.tile_pool(name="ptp", bufs=7))
        vscrp = ctx.enter_context(tc.tile_pool(name="vscrp", bufs=2))
        pcp = ctx.enter_context(tc.tile_pool(name="pcp", bufs=1))
        normp = ctx.enter_context(tc.tile_pool(name="normp", bufs=2))
        rdenp = ctx.enter_context(tc.tile_pool(name="rdenp", bufs=1))
        ysp = ctx.enter_context(tc.tile_pool(name="ysp", bufs=2))
        # PSUM: scores 2x[128,2,512]f32 (4 banks) + pv 1x[65,2,512]f32
        # (2 banks) + weave 2x[128,512]f32 (2 banks) = 8 banks exactly
        scp = ctx.enter_context(tc.tile_pool(name="scp", bufs=2, space="PSUM"))
        povp = ctx.enter_context(tc.tile_pool(name="povp", bufs=1, space="PSUM"))
        wvp = ctx.enter_context(tc.tile_pool(name="wvp", bufs=2, space="PSUM"))

        w_sb = {}
        for nm in ("q", "k", "v"):
            w_sb[nm] = consts.tile([128, EC, PSL], BF16, tag=f"w{nm}",
                                   name=f"w{nm}")
        wo_s = consts.tile([128, NPAIR, E], BF16, tag="wo", name="wo")
        b_sb = {}
        for nm in ("q", "k", "v"):
            b_sb[nm] = consts.tile([128, NPAIR], F32, tag=f"b{nm}",
                                   name=f"b{nm}")
        w_d = {"q": wq_d, "k": wk_d, "v": wv_d}
        b_d = {"q": bq_d, "k": bk_d, "v": bv_d}

        x_sb, x_d = {}, {"q": qT_d, "k": kT_d, "v": vT_d}
        for nm in ("q", "k", "v"):
            x_sb[nm] = xpool.tile([128, NCH, EC, TCH], BF16, tag=f"x{nm}",
                                  name=f"x{nm}")

        qTs = persist.tile([128, NPAIR, T], BF16, tag="qTs", name="qTs")
        kTs = persist.tile([128, NPAIR, S], BF16, tag="kTs", name="kTs")
        # [s-part, pair, st, head-in-pair, d | ones col at 64]
        vnat = persist.tile([128, NPAIR, ST, 2, 65], BF16, tag="vnat",
                            name="vnat")
        outn = persist.tile([128, NPAIR, T], BF16, tag="outn", name="outn")
        vt_sb = persist.tile([128, NPAIR, S], BF16, tag="vt", name="vt")

        def load_w(nm):
            # SWDGE queue: keeps the scalar HWDGE queue pure-x so the first
            # projection inputs land ~5us earlier
            nc.gpsimd.dma_start(w_sb[nm][:], w_d[nm][:])
            nc.gpsimd.dma_start(b_sb[nm][:], b_d[nm][:])

        def load_x(nm, c):
            nc.scalar.dma_start(x_sb[nm][:, c], x_d[nm][:, c])

        def proj_chunk(nm, p, c, on_act=False):
            """project x[nm] cols [c*512, (c+1)*512) for pair p."""
            dst = {"q": qTs, "k": kTs, "v": vt_sb}[nm]
            pp = wvp.tile([128, NB], F32, tag="wv", name="wv")
            for ec in range(EC):
                nc.tensor.matmul(
                    pp[:],
                    w_sb[nm][:, ec, p * 128:(p + 1) * 128],
                    x_sb[nm][:, c, ec, :],
                    start=(ec == 0), stop=(ec == EC - 1),
                )
            if on_act:
                nc.scalar.activation(dst[:, p, c * TCH:(c + 1) * TCH], pp[:],
                                     AF.Identity, bias=b_sb[nm][:, p:p + 1])
            else:
                nc.vector.tensor_scalar_add(
                    dst[:, p, c * TCH:(c + 1) * TCH], pp[:],
                    b_sb[nm][:, p:p + 1])

        def vtrans(p, c):
            """xbar-transpose vt chunk c of pair p into vnat st 4c..4c+3.
            On the sync queue: the scalar queue carries the 14.6 MB x/w
            stream and would delay these ~40us (FIFO)."""
            vs = vscrp.tile([128, 4, 128], BF16, tag="vs", name="vs")
            nc.sync.dma_start_transpose(
                vs[:], vt_sb[:, p, c * TCH:(c + 1) * TCH])
            nc.vector.tensor_copy(
                vnat[:, p, 4 * c:4 * c + 4, :, 0:64],
                vs[:].rearrange("s j (h d) -> s j h d", h=2))

        # ---- micro-op weave queue: each item costs <=~430ns of PE so no
        # st slot overflows (slot budget: pace 1.15us - scores .21 - pv .43)
        # Items carry completion keys; consumers call ensure(key) to drain
        # the queue far enough BEFORE emitting a dependent read (Tile does
        # not protect emission-order read-before-write).
        micro = []
        done_keys = {("q", 0, 0), ("k", 0, 0)}  # emitted in the head phase

        def pop_micro(budget=440):
            pe = 0
            while micro and pe < budget:
                fn, cost, key = micro.pop(0)
                fn()
                if key is not None:
                    done_keys.add(key)
                pe += cost

        def ensure(key):
            while micro and key not in done_keys:
                pop_micro(10000)

        def enqueue_chunk(nm, p, c):
            dst = {"q": qTs, "k": kTs, "v": vt_sb}[nm]
            box = {}

            def mk(i):
                def f():
                    if i == 0:
                        box["pp"] = wvp.tile([128, NB], F32, tag="wv",
                                             name="wv")
                    for ec in (2 * i, 2 * i + 1):
                        nc.tensor.matmul(
                            box["pp"][:],
                            w_sb[nm][:, ec, p * 128:(p + 1) * 128],
                            x_sb[nm][:, c, ec, :],
                            start=(ec == 0), stop=(ec == EC - 1),
                        )
                return f
            for i in range(4):
                micro.append((mk(i), 430, None))

            def evac():
                # ACT evacuation: ACT idles in the DMA-bound early phase
                # where the projections run; keeps DVE free for the mults
                nc.scalar.activation(dst[:, p, c * TCH:(c + 1) * TCH],
                                     box["pp"][:], AF.Identity,
                                     bias=b_sb[nm][:, p:p + 1])
            micro.append((evac, 20, (nm, p, c)))

        def enqueue_vt(p, c):
            micro.append((lambda: vtrans(p, c), 20, ("vt", p, c)))

        def enqueue_op(th, tt, tail=False):
            r0 = th * TCH + tt * 128
            box = {}

            def mk(k):
                def f():
                    box[k] = wvp.tile([128, NB], F32, tag="wv", name="wv")
                    for p in range(NPAIR):
                        nc.tensor.matmul(
                            box[k][:],
                            outn[:, p, r0:r0 + 128],
                            wo_s[:, p, k * NB:(k + 1) * NB],
                            start=(p == 0), stop=(p == NPAIR - 1),
                        )
                return f
            micro.append((mk(0), 430, None))
            micro.append((mk(1), 430, None))

            def fin():
                ys = ysp.tile([128, E], BF16, tag="ys", name="ys")
                if tail:
                    # tail: ACT is idle - split the two casts across engines
                    # and the stores across both spare DMA queues
                    nc.scalar.copy(ys[:, 0:NB], box[0][:])
                    nc.vector.tensor_copy(ys[:, NB:E], box[1][:])
                    eng = nc.gpsimd if tt % 2 == 0 else nc.sync
                    eng.dma_start(y_d[r0:r0 + 128, :], ys[:])
                else:
                    nc.vector.tensor_copy(ys[:, 0:NB], box[0][:])
                    nc.vector.tensor_copy(ys[:, NB:E], box[1][:])
                    nc.gpsimd.dma_start(y_d[r0:r0 + 128, :], ys[:])
            micro.append((fin, 20, None))

        # pending[0] = (p, th, pouts, ptl) of the previous block, whose last
        # LAG PVs + normalization are woven into the NEXT block's first
        # slots (avoids the tail PV burst delaying the next block's scores,
        # and pouts is freed within ~1.2us by a DVE evacuation to SBUF).
        pending = []

        def emit_pv(p, st, ptl, pouts):
            for h in range(2):
                nc.tensor.matmul(
                    pouts[:, h, :],
                    vnat[:, p, st, h, :],
                    ptl[st][:, h, :],
                    start=(st == 0), stop=(st == ST - 1),
                )

        def finish_pending(slot, direct=False):
            """Emit deferred tail work of the previous block at `slot` of
            the current block (2 PV st-tiles per slot; evac+norm after).
            direct=True (final block): skip the SBUF evacuation and
            normalize straight off the PSUM accumulator."""
            if not pending:
                return
            p, th, pouts, ptl = pending[0]
            t0 = th * TCH
            if slot < 2:
                for st in range(ST - LAG + 2 * slot,
                                min(ST, ST - LAG + 2 * slot + 2)):
                    emit_pv(p, st, ptl, pouts)
                return
            pending.pop(0)
            for st in range(ST - LAG + 4, ST):
                emit_pv(p, st, ptl, pouts)
            den = rdenp.tile([1, 2, TCH], F32, tag="den", name="den")
            if direct:
                # ACT handles the partition-shifted PSUM read
                nc.scalar.copy(den[:], pouts[64:65, :, :])
            else:
                # evacuate pouts (frees the PSUM banks for this block's PV)
                pc = pcp.tile([65, 2, TCH], F32, tag="pc", name="pc")
                nc.vector.tensor_copy(pc[:], pouts[:])
                # den to partition 0 first: custom-DVE ops cannot take
                # partition-shifted sources
                nc.vector.tensor_copy(den[:], pc[64:65, :, :])
            rden = rdenp.tile([1, 2, TCH], F32, tag="rden", name="rden")
            nc.vector.reciprocal_approx_fast(rden[:], den[:])
            for h in range(2):
                rb = normp.tile([64, TCH], F32, tag="rb", name="rb")
                nc.gpsimd.partition_broadcast(rb[:], rden[:, h, :])
                src = pouts if direct else pc
                if direct and h == 1:
                    po_s = pcp.tile([64, TCH], F32, tag="po", name="po")
                    nc.vector.tensor_copy(po_s[:], pouts[0:64, 1, :])
                    nc.vector.tensor_mul(
                        outn[64:128, p, t0:t0 + TCH], po_s[:], rb[:])
                else:
                    nc.vector.tensor_mul(
                        outn[64 * h:64 * h + 64, p, t0:t0 + TCH],
                        src[0:64, h, :], rb[:])

        def attention_block(p, th, inject=None, wbudget=440):
            t0 = th * TCH
            pouts = None  # allocated lazily at st == LAG, after the previous
            # generation's deferred evacuation (povp bufs=1 aliases memory)
            ptl = []
            btl = {}
            for st in range(ST):
                if st % 2 == 0:
                    if (th, p, st // 2) in pre_bias:
                        btl[st // 2] = pre_bias.pop((th, p, st // 2))
                    else:
                        bt = biasp.tile([128, 2, 2, TCH], BF16, tag="bias",
                                        name="bias")
                        nc.sync.dma_start(bt[:], bias_d[th, p, st // 2])
                        btl[st // 2] = bt
                ensure(("q", p, th))
                ensure(("k", p, st // 4))
                sc = scp.tile([128, 2, TCH], F32, tag="sc", name="sc")
                for h in range(2):
                    nc.tensor.matmul(
                        sc[:, h, :],
                        kTs[64 * h:64 * h + 64, p, st * 128:(st + 1) * 128],
                        qTs[64 * h:64 * h + 64, p, t0:t0 + TCH],
                        start=True, stop=True,
                        tile_position=(64 * h, 0),
                    )
                pt = ptp.tile([128, 2, TCH], BF16, tag="pt", name="pt")
                nc.scalar.activation(pt[:], sc[:], AF.Exp)
                nc.vector.tensor_mul(pt[:], pt[:], btl[st // 2][:, st % 2])
                ptl.append(pt)
                if st <= 2:
                    finish_pending(st)
                if inject and st in inject:
                    for item in inject[st]:
                        item()
                pop_micro(wbudget // 2 if st <= 2 else wbudget)
                if st >= LAG:
                    if pouts is None:
                        pouts = povp.tile([65, 2, TCH], F32, tag="acc",
                                          name="acc")
                    ensure(("vt", p, (st - LAG) // 4))
                    emit_pv(p, st - LAG, ptl, pouts)
            pending.append((p, th, pouts, ptl))

        def flush_all():
            while pending:
                for slot in range(3):
                    finish_pending(slot, direct=(len(pending) == 1))

        # ---------------- head: DMA issue order + minimal prep ------------
        nc.vector.memset(vnat[:, :, :, :, 64:65], 1.0)
        # prime the ACT exp table load (~2.7us) during the initial DMA wait
        dumm = consts.tile([1, 16], F32, tag="dumm", name="dumm")
        nc.vector.memset(dumm[:], 0.0)
        nc.scalar.activation(dumm[:], dumm[:], AF.Exp)
        # PE warm-up: ~50 dep-free N=128 matmuls (~5us) during the initial
        # DMA wait flips HAM to K=8/8 before the first projection
        wrm = consts.tile([128, NB], BF16, tag="wrm", name="wrm")
        nc.vector.memset(wrm[:], 0.0)
        for _ in range(9):
            wp = wvp.tile([128, NB], F32, tag="wv", name="wv")
            nc.tensor.matmul(wp[:], wrm[:, 0:128], wrm[:],
                             start=True, stop=True)
        load_w("q")
        load_w("k")
        load_w("v")
        load_x("q", 0)
        load_x("k", 0)
        # block-1's first bias tiles on the (otherwise pure-x) scalar queue,
        # right behind the data that gates the first scores
        pre_bias = {}
        for sp2 in (0, 1):
            bt = biasp.tile([128, 2, 2, TCH], BF16, tag="bias", name="bias")
            nc.scalar.dma_start(bt[:], bias_d[0, 0, sp2])
            pre_bias[(0, 0, sp2)] = bt
        load_x("k", 1)
        load_x("v", 0)
        load_x("k", 2)
        load_x("v", 1)
        load_x("k", 3)
        load_x("v", 2)
        load_x("v", 3)
        load_x("q", 1)
        load_x("q", 2)
        load_x("q", 3)
        nc.gpsimd.dma_start(wo_s[:], wo_d[:])

        proj_chunk("q", 0, 0, on_act=True)
        proj_chunk("k", 0, 0, on_act=True)

        # ---------------- micro-op weave schedule -------------------------
        for c in (1, 2, 3):
            enqueue_chunk("k", 0, c)
            enqueue_chunk("v", 0, c - 1)
            enqueue_vt(0, c - 1)
        enqueue_chunk("v", 0, 3)
        enqueue_vt(0, 3)
        enqueue_chunk("q", 0, 1)
        enqueue_chunk("q", 1, 0)
        for c in range(4):
            enqueue_chunk("k", 1, c)
            enqueue_chunk("v", 1, c)
            enqueue_vt(1, c)
        enqueue_chunk("q", 1, 1)
        enqueue_chunk("q", 0, 2)
        enqueue_chunk("q", 1, 2)
        enqueue_chunk("q", 0, 3)
        enqueue_chunk("q", 1, 3)

        def inject_ops(th):
            return {4: [lambda th=th: [enqueue_op(th, tt)
                                       for tt in range(4)]]}

        attention_block(0, 0, wbudget=1000)
        attention_block(0, 1, wbudget=1000)
        attention_block(1, 0, wbudget=900)
        attention_block(1, 1, inject=inject_ops(0))
        attention_block(0, 2, inject=inject_ops(1))
        attention_block(1, 2)
        attention_block(0, 3, inject=inject_ops(2))
        attention_block(1, 3)
        flush_all()
        while micro:
            pop_micro(10000)
        for tt in range(4):
            enqueue_op(3, tt, tail=True)
        while micro:
            pop_micro(10000)

    nc.compile()
    _MODULES[key] = nc
    return nc


def make_in_maps(query, key, value, spatial_bias, directional_bias,
                 key_padding_mask, attn_mask, Wq, bq, Wk, bk, Wv, bv, Wo, bo):
    scale = D ** -0.5

    def prep_x(x):
        # [T, E] -> [E, T] -> [128, NCH, EC, TCH] (8KB contiguous/partition)
        xt = np.ascontiguousarray(x.T, dtype=NPBF16)          # [E, T]
        return np.ascontiguousarray(
            xt.reshape(EC, 128, NCH, TCH).transpose(1, 2, 0, 3))

    qT = [prep_x(query[b]) for b in range(B)]
    kT = [prep_x(key[b]) for b in range(B)]
    vT = [prep_x(value[b]) for b in range(B)]
    pad_any = bool(np.any(key_padding_mask))
    in_maps = []
    for c in range(NCORES):
        b = c // 4
        h0 = (c % 4) * HPC
        sl = slice(h0 * D, (h0 + HPC) * D)
        bias = spatial_bias[b, h0:h0 + HPC].astype(np.float32) \
            + directional_bias[b, h0:h0 + HPC]
        bias += attn_mask[None]
        if pad_any:
            bias = np.where(key_padding_mask[b, None, None, :], -1e30, bias)
        np.exp(bias, out=bias)  # kernel applies bias multiplicatively
        # [h, T, S] -> [h, S, T] -> [NTH, NPAIR, sp2, 128, j, h, TCH]
        biasT = np.ascontiguousarray(bias.transpose(0, 2, 1), dtype=NPBF16)
        big = np.empty([NTH, NPAIR, ST // 2, 128, 2, 2, TCH], dtype=NPBF16)
        for p in range(NPAIR):
            for hip in range(2):
                # [S, T] -> (sp2, j, s128, th, tt) -> (th, sp2, s128, j, tt)
                arr = biasT[2 * p + hip].reshape(ST // 2, 2, 128, NTH, TCH)
                big[:, p, :, :, :, hip, :] = arr.transpose(3, 0, 2, 1, 4)
        in_maps.append({
            "qT": qT[b], "kT": kT[b], "vT": vT[b], "biasT": big,
            "wqT": np.ascontiguousarray(np.ascontiguousarray((Wq[sl, :].T * scale), dtype=NPBF16).reshape(EC, 128, PSL).transpose(1, 0, 2)),
            "wkT": np.ascontiguousarray(np.ascontiguousarray(Wk[sl, :].T, dtype=NPBF16).reshape(EC, 128, PSL).transpose(1, 0, 2)),
            "wvT": np.ascontiguousarray(np.ascontiguousarray(Wv[sl, :].T, dtype=NPBF16).reshape(EC, 128, PSL).transpose(1, 0, 2)),
            "woT": np.ascontiguousarray(np.ascontiguousarray(Wo[:, sl].T, dtype=NPBF16).reshape(NPAIR, 128, E).transpose(1, 0, 2)),
            "bq": (bq[sl] * scale).reshape(NPAIR, 128).T.astype(np.float32).copy(),
            "bk": bk[sl].reshape(NPAIR, 128).T.astype(np.float32).copy(),
            "bv": bv[sl].reshape(NPAIR, 128).T.astype(np.float32).copy(),
        })
    return in_maps


def _install_ntff_shim():
    """bass_utils' trace path imports antenv.axon_hooks, which this image
    lacks; synthesize it around trn_boot's ctypes NTFF hook."""
    import sys
    import types
    if "antenv.axon_hooks" in sys.modules:
        return
    try:
        import antenv
        from trn_agent_boot.trn_boot import _ntff_profile_via_ctypes
        hook = _ntff_profile_via_ctypes("/opt/axon/libaxon_pjrt.so")
        mod = types.ModuleType("antenv.axon_hooks")
        mod._hook = hook
        mod.get_axon_ntff_profile_hook = lambda: mod._hook
        mod.set_axon_ntff_profile_hook = lambda h: setattr(mod, "_hook", h)
        sys.modules["antenv.axon_hooks"] = mod
        antenv.axon_hooks = mod
    except Exception as exc:  # pragma: no cover
        print("ntff shim unavailable:", exc)


def kernel(**inputs):
    global LAST_RUN
    if os.environ.get("BASS_TRACE"):
        _install_ntff_shim()
    nc = build_module()
    in_maps = make_in_maps(**inputs)
    res = run_bass_kernel_spmd(
        nc, in_maps, core_ids=list(range(NCORES)),
        trace=bool(os.environ.get("BASS_TRACE")),
    )
    LAST_RUN = res
    bo = inputs["bo"]
    y = np.zeros((B, T, E), dtype=np.float64)
    for c in range(NCORES):
        y[c // 4] += res.results[c]["ypart"].astype(np.float64)
    y += bo
    return y.astype(np.float32)
